# revision 6
# baseline (speedup 1.0000x reference)
"""Trainium2 Bass kernel for LinearTransformerExpert.

Reference computation (per token n, 16 heads, head_dim 128, prefix len 8):
    q = x @ Wq.T ;  k = prefix @ Wk.T ;  v = prefix @ Wv.T
    scores[n,h,p] = q[n,h,:] . k[n,p,h,:] / sqrt(D)
    attn = softmax_p(scores);  out[n,h,:] = sum_p attn * v
    result = (out @ Wo.T) * sigmoid(x @ Wg.T + bg)

Strategy: data-parallel over the 8192 tokens across 8 NeuronCores (1024
tokens each), no collectives. All matmuls in bf16 on the PE (fp8
DoubleRow was measured slower per useful FLOP for the required 3-term
accuracy split). Structure tuned for PE occupancy:
  - host pre-tiles every operand so each DMA lands 2KB+ contiguous per
    partition (full DMA rate), with activations already transposed into
    stationary [contract, token] layout;
  - phase A computes q (kept resident in SBUF, bf16, folded softmax
    scale) AND the sigmoid gate (spilled to DRAM) so phase C only runs
    the output projection;
  - phase B fuses k/v projection + attention per (token-tile,
    prefix-pos); softmax/weighted-sum run on DVE/ACT hidden under the
    PE; the attention output is PE-transposed one tile behind the
    matmul stream so the PE never waits on the DVE chain;
  - phase C streams oT tiles back and multiplies by the spilled gate.
"""

import math

import numpy as np
import ml_dtypes

import concourse.bass as bass
import concourse.bacc as bacc
import concourse.mybir as mybir
from concourse import tile
from concourse.masks import make_identity

BF16 = mybir.dt.bfloat16
F32 = mybir.dt.float32
NPBF16 = ml_dtypes.bfloat16

N, P, C, H = 8192, 8, 2048, 16
D = C // H
NCORES = 8
NTOK = N // NCORES
NT = NTOK // 128
CT = C // 128
OH = 2
HPH = H // OH
SCALE = 1.0 / math.sqrt(D)


def _bcast(ap, n):
    return bass.AP(ap.tensor, ap.offset, list(ap.ap) + [[0, n]])


def build_nc(nt=NT):
    ntok = nt * 128
    nc = bacc.Bacc("TRN2", target_bir_lowering=False, debug=False,
                   num_devices=NCORES)

    xb = nc.dram_tensor("xb", [nt, 128, CT * 128], BF16, kind="ExternalInput")
    pb = nc.dram_tensor("pb", [nt, P, 128, CT * 128], BF16,
                        kind="ExternalInput")
    wq = nc.dram_tensor("wq", [128, CT * C], BF16, kind="ExternalInput")
    wk = nc.dram_tensor("wk", [128, CT * C], BF16, kind="ExternalInput")
    wv = nc.dram_tensor("wv", [128, CT * C], BF16, kind="ExternalInput")
    wg = nc.dram_tensor("wg", [128, CT * C], BF16, kind="ExternalInput")
    wo = nc.dram_tensor("wo", [128, CT * C], BF16, kind="ExternalInput")
    bgs = nc.dram_tensor("bgs", [1, C], BF16, kind="ExternalInput")
    out = nc.dram_tensor("out", [ntok, C], F32, kind="ExternalOutput")

    g_spill = nc.dram_tensor("g_spill", [nt, 128, C], BF16)
    ot_spill = nc.dram_tensor("ot_spill", [nt, 128, CT * 128], BF16)

    def mm1(psum, stat, mov, mov_off, width, tail=None):
        for cch in range(width // 512):
            sl = slice(cch * 512, (cch + 1) * 512)
            msl = slice(mov_off + cch * 512, mov_off + (cch + 1) * 512)
            for t in range(CT):
                last = (t == CT - 1 and tail is None)
                nc.tensor.matmul(psum[:, sl], stat[:, t, :], mov[:, t, msl],
                                 start=(t == 0), stop=last)
            if tail is not None:
                tail(sl, msl)

    with tile.TileContext(nc) as tc:
        with tc.tile_pool(name="qres", bufs=1) as qres_pool:
            q_res = qres_pool.tile([128, nt * C], BF16, tag="qres")
            q_v = q_res[:].rearrange("p (j o) -> p j o", j=nt)

            # ---------- Phase A1: gate = sigmoid(x@Wg.T + bg) -> spill ----------
            with tc.tile_pool(name="paw1", bufs=1) as paw1, \
                 tc.tile_pool(name="pax1", bufs=2) as pax1, \
                 tc.tile_pool(name="pag", bufs=2) as pag, \
                 tc.tile_pool(name="pac", bufs=1) as pac, \
                 tc.tile_pool(name="paz", bufs=2, space="PSUM") as paz:
                wg_sb = paw1.tile([128, CT * C], BF16, tag="wg")
                nc.sync.dma_start(out=wg_sb[:], in_=wg[:])
                wg_v = wg_sb[:].rearrange("p (t o) -> p t o", t=CT)
                ones_sb = pac.tile([1, 128], BF16, tag="ones")
                nc.vector.memset(ones_sb[:], 1.0)
                bg_sb = pac.tile([1, C], BF16, tag="bgs")
                nc.sync.dma_start(out=bg_sb[:], in_=bgs[:])
                for j in range(nt):
                    xb_t = pax1.tile([128, CT * 128], BF16, tag="xb")
                    nc.sync.dma_start(out=xb_t[:], in_=xb[j])
                    xb_v = xb_t[:].rearrange("p (t n) -> p t n", t=CT)
                    for hf in range(OH):
                        o0 = hf * (C // OH)

                        def bias_tail(sl, msl):
                            nc.tensor.matmul(zp[:, sl], ones_sb[:],
                                             bg_sb[:, msl],
                                             start=False, stop=True)
                        zp = paz.tile([128, C // OH], F32, tag="zp")
                        mm1(zp, xb_v, wg_v, o0, C // OH, tail=bias_tail)
                        gb = pag.tile([128, C // OH], BF16, tag="gb")
                        nc.scalar.activation(
                            gb[:], zp[:],
                            mybir.ActivationFunctionType.Sigmoid)
                        nc.sync.dma_start(
                            out=g_spill[j, :, o0:o0 + C // OH], in_=gb[:])

            # ---- Phase A2: q = x@Wq.T (resident), wk prefetched under it ----
            pbwk = tc.alloc_tile_pool(name="pbwk", bufs=1)
            wk_sb = pbwk.tile([128, CT * C], BF16, tag="wk")
            with tc.tile_pool(name="paw2", bufs=1) as paw2, \
                 tc.tile_pool(name="pax2", bufs=2) as pax2, \
                 tc.tile_pool(name="paq", bufs=2, space="PSUM") as paq:
                wq_sb = paw2.tile([128, CT * C], BF16, tag="wq")
                nc.sync.dma_start(out=wq_sb[:], in_=wq[:])
                nc.sync.dma_start(out=wk_sb[:], in_=wk[:])
                wq_v = wq_sb[:].rearrange("p (t o) -> p t o", t=CT)
                for j in range(nt):
                    xb_t = pax2.tile([128, CT * 128], BF16, tag="xb")
                    nc.sync.dma_start(out=xb_t[:], in_=xb[j])
                    xb_v = xb_t[:].rearrange("p (t n) -> p t n", t=CT)
                    for hf in range(OH):
                        o0 = hf * (C // OH)
                        qp = paq.tile([128, C // OH], F32, tag="qp")
                        mm1(qp, xb_v, wq_v, o0, C // OH)
                        nc.scalar.mul(q_v[:, j, o0:o0 + C // OH], qp[:], SCALE)

            # ------- Phase B: k/v projection + attention, fused -------
            with tc.tile_pool(name="pbw", bufs=1) as pbw, \
                 tc.tile_pool(name="pbp", bufs=2) as pbp, \
                 tc.tile_pool(name="pbkv", bufs=2) as pbkv, \
                 tc.tile_pool(name="pbacc", bufs=1) as pbacc, \
                 tc.tile_pool(name="pbsc", bufs=1) as pbsc, \
                 tc.tile_pool(name="pbsm", bufs=3) as pbsm, \
                 tc.tile_pool(name="pbo", bufs=2) as pbo, \
                 tc.tile_pool(name="pbot", bufs=1) as pbot, \
                 tc.tile_pool(name="pbi", bufs=1) as pbi, \
                 tc.tile_pool(name="pbpsk", bufs=2, space="PSUM") as pbpsk, \
                 tc.tile_pool(name="pbpsv", bufs=1, space="PSUM") as pbpsv, \
                 tc.tile_pool(name="pbpst", bufs=2, space="PSUM") as pbpst:
                wv_sb = pbw.tile([128, CT * C], BF16, tag="wv")
                nc.sync.dma_start(out=wv_sb[:], in_=wv[:])
                wk_v = wk_sb[:].rearrange("p (t o) -> p t o", t=CT)
                wv_v = wv_sb[:].rearrange("p (t o) -> p t o", t=CT)
                ident = pbi.tile([128, 128], BF16, tag="ident")
                make_identity(nc, ident[:])

                pending = []

                def flush_pending():
                    ob_p, jj = pending.pop()
                    obT = pbot.tile([128, CT * 128], BF16, tag="obT")
                    for t in range(CT):
                        tp = pbpst.tile([128, 128], BF16, tag="tp")
                        nc.tensor.transpose(
                            tp[:], ob_p[:, t * 128:(t + 1) * 128], ident[:])
                        nc.scalar.copy(obT[:, t * 128:(t + 1) * 128], tp[:])
                    nc.sync.dma_start(out=ot_spill[jj], in_=obT[:])

                for j in range(nt):
                    O = pbacc.tile([128, C], F32, tag="O")
                    s_den = pbsm.tile([128, H], F32, tag="sden")
                    for p in range(P):
                        pb_t = pbp.tile([128, CT * 128], BF16, tag="pb")
                        nc.sync.dma_start(out=pb_t[:], in_=pb[j, p])
                        pb_v = pb_t[:].rearrange("p (t n) -> p t n", t=CT)
                        for hf in range(OH):
                            o0 = hf * (C // OH)
                            kp = pbpsk.tile([128, C // OH], F32, tag="kp")
                            mm1(kp, pb_v, wk_v, o0, C // OH)
                            vp = pbpsv.tile([128, C // OH], F32, tag="vp")
                            mm1(vp, pb_v, wv_v, o0, C // OH)
                            kb = pbkv.tile([128, C // OH], BF16, tag="kb")
                            nc.scalar.copy(kb[:], kp[:])
                            vb = pbkv.tile([128, C // OH], BF16, tag="vb")
                            nc.vector.tensor_copy(vb[:], vp[:])
                            prod = pbsc.tile([128, C // OH], BF16, tag="prod")
                            nc.vector.tensor_mul(
                                prod[:], q_v[:, j, o0:o0 + C // OH], kb[:])
                            sc = pbsm.tile([128, HPH], F32, tag="sc")
                            nc.vector.tensor_reduce(
                                sc[:],
                                prod[:].rearrange("p (h d) -> p h d", d=D),
                                mybir.AxisListType.X, mybir.AluOpType.add)
                            ee = pbsm.tile([128, HPH], F32, tag="ee")
                            nc.scalar.activation(
                                ee[:], sc[:], mybir.ActivationFunctionType.Exp)
                            s_sl = s_den[:, hf * HPH:(hf + 1) * HPH]
                            if p == 0:
                                nc.vector.tensor_copy(s_sl, ee[:])
                            else:
                                nc.vector.tensor_add(s_sl, s_sl, ee[:])
                            O_v = O[:, o0:o0 + C // OH].rearrange(
                                "p (h d) -> p h d", d=D)
                            v_v = vb[:].rearrange("p (h d) -> p h d", d=D)
                            e_b = _bcast(ee[:], D)
                            if p == 0:
                                nc.vector.tensor_tensor(O_v, v_v, e_b,
                                                        mybir.AluOpType.mult)
                            else:
                                tmp = pbsc.tile([128, C // OH], F32,
                                                tag="tmp")
                                tmp_v = tmp[:].rearrange("p (h d) -> p h d",
                                                         d=D)
                                nc.vector.tensor_tensor(tmp_v, v_v, e_b,
                                                        mybir.AluOpType.mult)
                                nc.vector.tensor_add(
                                    O[:, o0:o0 + C // OH],
                                    O[:, o0:o0 + C // OH], tmp[:])
                        if p == 2 and pending:
                            flush_pending()
                    s_inv = pbsm.tile([128, H], F32, tag="sinv")
                    nc.vector.reciprocal(s_inv[:], s_den[:])
                    ob = pbo.tile([128, C], BF16, tag="ob")
                    nc.vector.tensor_tensor(
                        ob[:].rearrange("p (h d) -> p h d", d=D),
                        O[:].rearrange("p (h d) -> p h d", d=D),
                        _bcast(s_inv[:], D), mybir.AluOpType.mult)
                    pending.append((ob, j))
                flush_pending()
            pbwk.release()

            # ---------- Phase C: result = (o @ Wo.T) * g ----------
            with tc.tile_pool(name="pcw", bufs=1) as pcw, \
                 tc.tile_pool(name="pco", bufs=2) as pco, \
                 tc.tile_pool(name="pcg", bufs=2) as pcg, \
                 tc.tile_pool(name="pcf", bufs=2) as pcf, \
                 tc.tile_pool(name="pcps", bufs=2, space="PSUM") as pcps:
                wo_sb = pcw.tile([128, CT * C], BF16, tag="wo")
                nc.sync.dma_start(out=wo_sb[:], in_=wo[:])
                wo_v = wo_sb[:].rearrange("p (t o) -> p t o", t=CT)
                for j in range(nt):
                    ot_t = pco.tile([128, CT * 128], BF16, tag="ot")
                    nc.sync.dma_start(out=ot_t[:], in_=ot_spill[j])
                    gs_t = pcg.tile([128, C], BF16, tag="gs")
                    nc.sync.dma_start(out=gs_t[:], in_=g_spill[j])
                    ot_v = ot_t[:].rearrange("p (t n) -> p t n", t=CT)
                    for hq in range(4):
                        o0 = hq * 512
                        fp = pcps.tile([128, 512], F32, tag="fp")
                        mm1(fp, ot_v, wo_v, o0, 512)
                        fb = pcf.tile([128, 512], F32, tag="fb")
                        nc.vector.tensor_mul(fb[:], fp[:],
                                             gs_t[:, o0:o0 + 512])
                        nc.sync.dma_start(
                            out=out[j * 128:(j + 1) * 128, o0:o0 + 512],
                            in_=fb[:])

    nc.compile()
    return nc


_NC_CACHE = {}


def _get_nc(nt=NT):
    if nt not in _NC_CACHE:
        _NC_CACHE[nt] = build_nc(nt)
    return _NC_CACHE[nt]


def _tile_w(w):
    wt = np.asarray(w, dtype=np.float32).T
    wt = wt.reshape(CT, 128, C).transpose(1, 0, 2).reshape(128, CT * C)
    return np.ascontiguousarray(wt).astype(NPBF16)


def prep_core_inputs(x, prefix, Wq, Wk, Wv, Wo, Wg, bg):
    x = np.asarray(x, dtype=np.float32)
    prefix = np.asarray(prefix, dtype=np.float32)
    wqt, wkt, wvt = _tile_w(Wq), _tile_w(Wk), _tile_w(Wv)
    wgt, wot = _tile_w(Wg), _tile_w(Wo)
    bg_s = np.ascontiguousarray(
        np.asarray(bg, dtype=np.float32).reshape(1, C)).astype(NPBF16)
    in_maps = []
    for c in range(NCORES):
        sl = slice(c * NTOK, (c + 1) * NTOK)
        xt = x[sl].reshape(NT, 128, CT, 128).transpose(0, 3, 2, 1)
        xt = np.ascontiguousarray(xt).reshape(NT, 128, CT * 128).astype(NPBF16)
        pt = prefix[sl].reshape(NT, 128, P, CT, 128).transpose(0, 2, 4, 3, 1)
        pt = np.ascontiguousarray(pt).reshape(NT, P, 128,
                                              CT * 128).astype(NPBF16)
        in_maps.append({"xb": xt, "pb": pt, "wq": wqt, "wk": wkt, "wv": wvt,
                        "wg": wgt, "wo": wot, "bgs": bg_s})
    return in_maps


def kernel(x, prefix, Wq, Wk, Wv, Wo, Wg, bg):
    from concourse.bass_utils import run_bass_kernel_spmd
    in_maps = prep_core_inputs(x, prefix, np.asarray(Wq), np.asarray(Wk),
                               np.asarray(Wv), np.asarray(Wo), np.asarray(Wg),
                               np.asarray(bg))
    nc = _get_nc()
    res = run_bass_kernel_spmd(nc, in_maps, core_ids=list(range(NCORES)))
    return np.concatenate([res.results[c]["out"] for c in range(NCORES)],
                          axis=0)


# revision 7
# speedup vs baseline: 1.1732x; 1.1732x over previous
"""Trainium2 Bass kernel for LinearTransformerExpert.

Reference computation (per token n, 16 heads, head_dim 128, prefix len 8):
    q = x @ Wq.T ;  k = prefix @ Wk.T ;  v = prefix @ Wv.T
    scores[n,h,p] = q[n,h,:] . k[n,p,h,:] / sqrt(D)
    attn = softmax_p(scores);  out[n,h,:] = sum_p attn * v
    result = (out @ Wo.T) * sigmoid(x @ Wg.T + bg)

Strategy: data-parallel over the 8192 tokens across 8 NeuronCores (1024
tokens each), no collectives. All matmuls run in bf16 on the PE with the
activations as the stationary operand (fp8 DoubleRow was measured slower
per useful FLOP once the 3-term hi/lo split needed for the accuracy gate
is priced in: 3 fp8 passes = 1.5x bf16 time). Structure is tuned for PE
occupancy:
  - the host pre-tiles every operand so each DMA lands 2KB+ contiguous
    per partition (full DMA rate), with activations already transposed
    into the stationary [contract, token] layout;
  - phase A computes q (kept resident in SBUF in bf16 with the softmax
    scale folded in) AND the sigmoid gate (spilled to DRAM), so phase C
    only runs the output projection;
  - phase B fuses k/v projection + attention per (token-tile,
    prefix-pos); softmax/weighted-sum run on DVE/ACT hidden under the
    PE; the attention output is PE-transposed one token-tile behind the
    matmul stream so the PE never waits on the DVE chain;
  - phase C streams the transposed attention output back and multiplies
    by the spilled gate.
"""

import math

import numpy as np
import ml_dtypes

import concourse.bass as bass
import concourse.bacc as bacc
import concourse.mybir as mybir
from concourse import tile
from concourse.masks import make_identity

BF16 = mybir.dt.bfloat16
F32 = mybir.dt.float32
NPBF16 = ml_dtypes.bfloat16

N, P, C, H = 8192, 8, 2048, 16
D = C // H
NCORES = 8
NTOK = N // NCORES
NT = NTOK // 128
CT = C // 128
OH = 2
HPH = H // OH
SCALE = 1.0 / math.sqrt(D)


def _bcast(ap, n):
    return bass.AP(ap.tensor, ap.offset, list(ap.ap) + [[0, n]])


def build_nc(nt=NT):
    ntok = nt * 128
    nc = bacc.Bacc("TRN2", target_bir_lowering=False, debug=False,
                   num_devices=NCORES)

    xb = nc.dram_tensor("xb", [nt, 128, CT * 128], BF16, kind="ExternalInput")
    pb = nc.dram_tensor("pb", [nt, P, 128, CT * 128], BF16,
                        kind="ExternalInput")
    wq = nc.dram_tensor("wq", [128, CT * C], BF16, kind="ExternalInput")
    wk = nc.dram_tensor("wk", [128, CT * C], BF16, kind="ExternalInput")
    wv = nc.dram_tensor("wv", [128, CT * C], BF16, kind="ExternalInput")
    wg = nc.dram_tensor("wg", [128, CT * C], BF16, kind="ExternalInput")
    wo = nc.dram_tensor("wo", [128, CT * C], BF16, kind="ExternalInput")
    bgs = nc.dram_tensor("bgs", [1, C], BF16, kind="ExternalInput")
    out = nc.dram_tensor("out", [ntok, C], F32, kind="ExternalOutput")

    g_spill = nc.dram_tensor("g_spill", [nt, 128, C], BF16)
    ot_spill = nc.dram_tensor("ot_spill", [nt, 128, CT * 128], BF16)

    def mm1(psum, stat, mov, mov_off, width, tail=None):
        for cch in range(width // 512):
            sl = slice(cch * 512, (cch + 1) * 512)
            msl = slice(mov_off + cch * 512, mov_off + (cch + 1) * 512)
            for t in range(CT):
                last = (t == CT - 1 and tail is None)
                nc.tensor.matmul(psum[:, sl], stat[:, t, :], mov[:, t, msl],
                                 start=(t == 0), stop=last)
            if tail is not None:
                tail(sl, msl)

    with tile.TileContext(nc) as tc:
        with tc.tile_pool(name="qres", bufs=1) as qres_pool:
            q_res = qres_pool.tile([128, nt * C], BF16, tag="qres")
            q_v = q_res[:].rearrange("p (j o) -> p j o", j=nt)

            # ---------- Phase A: q (resident) + gate (spilled) ----------
            with tc.tile_pool(name="paw", bufs=1) as paw, \
                 tc.tile_pool(name="pax", bufs=2) as pax, \
                 tc.tile_pool(name="pag", bufs=2) as pag, \
                 tc.tile_pool(name="pac", bufs=1) as pac, \
                 tc.tile_pool(name="paq", bufs=2, space="PSUM") as paq, \
                 tc.tile_pool(name="paz", bufs=2, space="PSUM") as paz:
                wq_sb = paw.tile([128, CT * C], BF16, tag="wq")
                wg_sb = paw.tile([128, CT * C], BF16, tag="wg")
                nc.sync.dma_start(out=wq_sb[:], in_=wq[:])
                nc.sync.dma_start(out=wg_sb[:], in_=wg[:])
                wq_v = wq_sb[:].rearrange("p (t o) -> p t o", t=CT)
                wg_v = wg_sb[:].rearrange("p (t o) -> p t o", t=CT)
                ones_sb = pac.tile([1, 128], BF16, tag="ones")
                nc.vector.memset(ones_sb[:], 1.0)
                bg_sb = pac.tile([1, C], BF16, tag="bgs")
                nc.sync.dma_start(out=bg_sb[:], in_=bgs[:])

                for j in range(nt):
                    xb_t = pax.tile([128, CT * 128], BF16, tag="xb")
                    nc.sync.dma_start(out=xb_t[:], in_=xb[j])
                    xb_v = xb_t[:].rearrange("p (t n) -> p t n", t=CT)
                    for hf in range(OH):
                        o0 = hf * (C // OH)
                        qp = paq.tile([128, C // OH], F32, tag="qp")
                        mm1(qp, xb_v, wq_v, o0, C // OH)
                        nc.scalar.mul(q_v[:, j, o0:o0 + C // OH], qp[:], SCALE)

                        def bias_tail(sl, msl):
                            nc.tensor.matmul(zp[:, sl], ones_sb[:],
                                             bg_sb[:, msl],
                                             start=False, stop=True)
                        zp = paz.tile([128, C // OH], F32, tag="zp")
                        mm1(zp, xb_v, wg_v, o0, C // OH, tail=bias_tail)
                        gb = pag.tile([128, C // OH], BF16, tag="gb")
                        nc.scalar.activation(
                            gb[:], zp[:],
                            mybir.ActivationFunctionType.Sigmoid)
                        nc.sync.dma_start(
                            out=g_spill[j, :, o0:o0 + C // OH], in_=gb[:])

            # ------- Phase B: k/v projection + attention, fused -------
            with tc.tile_pool(name="pbw", bufs=1) as pbw, \
                 tc.tile_pool(name="pbp", bufs=2) as pbp, \
                 tc.tile_pool(name="pbkv", bufs=2) as pbkv, \
                 tc.tile_pool(name="pbacc", bufs=1) as pbacc, \
                 tc.tile_pool(name="pbsc", bufs=1) as pbsc, \
                 tc.tile_pool(name="pbsm", bufs=3) as pbsm, \
                 tc.tile_pool(name="pbo", bufs=2) as pbo, \
                 tc.tile_pool(name="pbot", bufs=1) as pbot, \
                 tc.tile_pool(name="pbi", bufs=1) as pbi, \
                 tc.tile_pool(name="pbpsk", bufs=2, space="PSUM") as pbpsk, \
                 tc.tile_pool(name="pbpsv", bufs=1, space="PSUM") as pbpsv, \
                 tc.tile_pool(name="pbpst", bufs=2, space="PSUM") as pbpst:
                wk_sb = pbw.tile([128, CT * C], BF16, tag="wk")
                wv_sb = pbw.tile([128, CT * C], BF16, tag="wv")
                nc.sync.dma_start(out=wk_sb[:], in_=wk[:])
                nc.sync.dma_start(out=wv_sb[:], in_=wv[:])
                wk_v = wk_sb[:].rearrange("p (t o) -> p t o", t=CT)
                wv_v = wv_sb[:].rearrange("p (t o) -> p t o", t=CT)
                ident = pbi.tile([128, 128], BF16, tag="ident")
                make_identity(nc, ident[:])

                pending = []

                def flush_pending():
                    ob_p, jj = pending.pop()
                    obT = pbot.tile([128, CT * 128], BF16, tag="obT")
                    for t in range(CT):
                        tp = pbpst.tile([128, 128], BF16, tag="tp")
                        nc.tensor.transpose(
                            tp[:], ob_p[:, t * 128:(t + 1) * 128], ident[:])
                        nc.scalar.copy(obT[:, t * 128:(t + 1) * 128], tp[:])
                    nc.sync.dma_start(out=ot_spill[jj], in_=obT[:])

                for j in range(nt):
                    O = pbacc.tile([128, C], F32, tag="O")
                    s_den = pbsm.tile([128, H], F32, tag="sden")
                    for p in range(P):
                        pb_t = pbp.tile([128, CT * 128], BF16, tag="pb")
                        nc.sync.dma_start(out=pb_t[:], in_=pb[j, p])
                        pb_v = pb_t[:].rearrange("p (t n) -> p t n", t=CT)
                        for hf in range(OH):
                            o0 = hf * (C // OH)
                            kp = pbpsk.tile([128, C // OH], F32, tag="kp")
                            mm1(kp, pb_v, wk_v, o0, C // OH)
                            vp = pbpsv.tile([128, C // OH], F32, tag="vp")
                            mm1(vp, pb_v, wv_v, o0, C // OH)
                            kb = pbkv.tile([128, C // OH], BF16, tag="kb")
                            nc.scalar.copy(kb[:], kp[:])
                            vb = pbkv.tile([128, C // OH], BF16, tag="vb")
                            nc.vector.tensor_copy(vb[:], vp[:])
                            prod = pbsc.tile([128, C // OH], BF16, tag="prod")
                            nc.vector.tensor_mul(
                                prod[:], q_v[:, j, o0:o0 + C // OH], kb[:])
                            sc = pbsm.tile([128, HPH], F32, tag="sc")
                            nc.vector.tensor_reduce(
                                sc[:],
                                prod[:].rearrange("p (h d) -> p h d", d=D),
                                mybir.AxisListType.X, mybir.AluOpType.add)
                            ee = pbsm.tile([128, HPH], F32, tag="ee")
                            nc.scalar.activation(
                                ee[:], sc[:], mybir.ActivationFunctionType.Exp)
                            s_sl = s_den[:, hf * HPH:(hf + 1) * HPH]
                            if p == 0:
                                nc.vector.tensor_copy(s_sl, ee[:])
                            else:
                                nc.vector.tensor_add(s_sl, s_sl, ee[:])
                            O_v = O[:, o0:o0 + C // OH].rearrange(
                                "p (h d) -> p h d", d=D)
                            v_v = vb[:].rearrange("p (h d) -> p h d", d=D)
                            e_b = _bcast(ee[:], D)
                            if p == 0:
                                nc.vector.tensor_tensor(O_v, v_v, e_b,
                                                        mybir.AluOpType.mult)
                            else:
                                tmp = pbsc.tile([128, C // OH], F32,
                                                tag="tmp")
                                tmp_v = tmp[:].rearrange("p (h d) -> p h d",
                                                         d=D)
                                nc.vector.tensor_tensor(tmp_v, v_v, e_b,
                                                        mybir.AluOpType.mult)
                                nc.vector.tensor_add(
                                    O[:, o0:o0 + C // OH],
                                    O[:, o0:o0 + C // OH], tmp[:])
                        if p == 2 and pending:
                            flush_pending()
                    s_inv = pbsm.tile([128, H], F32, tag="sinv")
                    nc.vector.reciprocal(s_inv[:], s_den[:])
                    ob = pbo.tile([128, C], BF16, tag="ob")
                    nc.vector.tensor_tensor(
                        ob[:].rearrange("p (h d) -> p h d", d=D),
                        O[:].rearrange("p (h d) -> p h d", d=D),
                        _bcast(s_inv[:], D), mybir.AluOpType.mult)
                    pending.append((ob, j))
                flush_pending()

            # ---------- Phase C: result = (o @ Wo.T) * g ----------
            with tc.tile_pool(name="pcw", bufs=1) as pcw, \
                 tc.tile_pool(name="pco", bufs=2) as pco, \
                 tc.tile_pool(name="pcg", bufs=2) as pcg, \
                 tc.tile_pool(name="pcf", bufs=2) as pcf, \
                 tc.tile_pool(name="pcps", bufs=2, space="PSUM") as pcps:
                wo_sb = pcw.tile([128, CT * C], BF16, tag="wo")
                nc.sync.dma_start(out=wo_sb[:], in_=wo[:])
                wo_v = wo_sb[:].rearrange("p (t o) -> p t o", t=CT)
                for j in range(nt):
                    ot_t = pco.tile([128, CT * 128], BF16, tag="ot")
                    nc.sync.dma_start(out=ot_t[:], in_=ot_spill[j])
                    gs_t = pcg.tile([128, C], BF16, tag="gs")
                    nc.sync.dma_start(out=gs_t[:], in_=g_spill[j])
                    ot_v = ot_t[:].rearrange("p (t n) -> p t n", t=CT)
                    for hq in range(4):
                        o0 = hq * 512
                        fp = pcps.tile([128, 512], F32, tag="fp")
                        mm1(fp, ot_v, wo_v, o0, 512)
                        fb = pcf.tile([128, 512], F32, tag="fb")
                        nc.vector.tensor_mul(fb[:], fp[:],
                                             gs_t[:, o0:o0 + 512])
                        nc.sync.dma_start(
                            out=out[j * 128:(j + 1) * 128, o0:o0 + 512],
                            in_=fb[:])

    nc.compile()
    return nc


_NC_CACHE = {}


def _get_nc(nt=NT):
    if nt not in _NC_CACHE:
        _NC_CACHE[nt] = build_nc(nt)
    return _NC_CACHE[nt]


def _tile_w(w):
    wt = np.asarray(w, dtype=np.float32).T
    wt = wt.reshape(CT, 128, C).transpose(1, 0, 2).reshape(128, CT * C)
    return np.ascontiguousarray(wt).astype(NPBF16)


def prep_core_inputs(x, prefix, Wq, Wk, Wv, Wo, Wg, bg):
    x = np.asarray(x, dtype=np.float32)
    prefix = np.asarray(prefix, dtype=np.float32)
    wqt, wkt, wvt = _tile_w(Wq), _tile_w(Wk), _tile_w(Wv)
    wgt, wot = _tile_w(Wg), _tile_w(Wo)
    bg_s = np.ascontiguousarray(
        np.asarray(bg, dtype=np.float32).reshape(1, C)).astype(NPBF16)
    in_maps = []
    for c in range(NCORES):
        sl = slice(c * NTOK, (c + 1) * NTOK)
        xt = x[sl].reshape(NT, 128, CT, 128).transpose(0, 3, 2, 1)
        xt = np.ascontiguousarray(xt).reshape(NT, 128, CT * 128).astype(NPBF16)
        pt = prefix[sl].reshape(NT, 128, P, CT, 128).transpose(0, 2, 4, 3, 1)
        pt = np.ascontiguousarray(pt).reshape(NT, P, 128,
                                              CT * 128).astype(NPBF16)
        in_maps.append({"xb": xt, "pb": pt, "wq": wqt, "wk": wkt, "wv": wvt,
                        "wg": wgt, "wo": wot, "bgs": bg_s})
    return in_maps


def kernel(x, prefix, Wq, Wk, Wv, Wo, Wg, bg):
    from concourse.bass_utils import run_bass_kernel_spmd
    in_maps = prep_core_inputs(x, prefix, np.asarray(Wq), np.asarray(Wk),
                               np.asarray(Wv), np.asarray(Wo), np.asarray(Wg),
                               np.asarray(bg))
    nc = _get_nc()
    res = run_bass_kernel_spmd(nc, in_maps, core_ids=list(range(NCORES)))
    return np.concatenate([res.results[c]["out"] for c in range(NCORES)],
                          axis=0)
